# revision 22
# baseline (speedup 1.0000x reference)
"""Trainium2 Bass kernel for nn_CausalBoostNet (gnn_message_passing).

Strategy
--------
Data-parallel over batch B=4096 across 8 NeuronCores (512 rows each). All
small parameters are replicated. The whole network is algebraically folded
on the host so the device executes, per 64-row batch tile:

  round 0:  X1 = x (*) wn1, A1 = x (*) wn2             (rank-1, DVE, bf16)
            Y0 = W^T @ X1  (+ A1 via identity matmul)  (PE)
            pre0 = relu(Y0 + A1)                       (ACT, -> bf16)
  T2:       pre0 [j,(b,h)] -> stacked [(b%2,h), (b/2,j)] (PE transpose)
  round 1:  XA = pre0 @ blockdiag[W_x2|W_a2]           (fused transpose-matmul,
            Y1 = W^T @ X2 (+ A2), pre1 = relu(.)        K=128, both b-parities)
  head:     logits = relu(pre1 . p1f + bt) @ p2w + p2b (PE, two tiles at once)

where wn1 = wn @ (agg_w0 @ u1b0), wn2 = wn @ u1a0, W_x2 = u2_0 @ (agg_w1 @ u1b1),
W_a2 = u2_0 @ u1a1, p1f[j,hp,o] = sum_h u2_1[hp,h] p1_w[(j,h),o], etc.
All per-feature biases (zeros in this problem's init) are folded into
rank-1 addends / per-partition activation biases; the emission of the
(tensor) bias adds is skipped when the host-folded values are exactly zero.
"""

import numpy as np
import ml_dtypes

import concourse.bacc as bacc
import concourse.bass as bass
import concourse.mybir as mybir
import concourse.tile as tile
from concourse import bass_utils, masks

# problem constants
B, D, H, C = 4096, 256, 64, 10
NCORES = 8
BS = B // NCORES          # per-core batch shard
PT = 64                   # batch tile
NT = BS // PT
TAU = 0.3

F32 = mybir.dt.float32
BF16 = mybir.dt.bfloat16
RELU = mybir.ActivationFunctionType.Relu
IDENT = mybir.ActivationFunctionType.Identity
MULT = mybir.AluOpType.mult
ADD = mybir.AluOpType.add


# ----------------------------------------------------------------- host math
def _get_W_np(A_raw, priority, edge_logits, tau=TAU):
    k = priority.shape[0]
    d = A_raw.shape[0]
    A = A_raw - A_raw.max(axis=1, keepdims=True)
    A = np.exp(A)
    A = A / A.sum(axis=1, keepdims=True)
    direction = 1.0 / (1.0 + np.exp(-(priority[None, :] - priority[:, None]) / tau))
    edges = (edge_logits > 0.0).astype(A.dtype)
    S = edges * direction * (1.0 - np.eye(k, dtype=A.dtype))
    W = (A @ S @ A.T) * (1.0 - np.eye(d, dtype=A.dtype))
    return W


def _fold(inp):
    f32 = lambda a: np.asarray(a, dtype=np.float32)
    wn, bn = f32(inp["wn"]), f32(inp["bn"])
    W = _get_W_np(f32(inp["A_raw"]), f32(inp["priority"]), f32(inp["edge_logits"]))
    agg_w, agg_b = f32(inp["agg_w"]), f32(inp["agg_b"])
    upd_w1, upd_b1 = f32(inp["upd_w1"]), f32(inp["upd_b1"])
    upd_w2, upd_b2 = f32(inp["upd_w2"]), f32(inp["upd_b2"])
    p1_w, p1_b = f32(inp["p1_w"]), f32(inp["p1_b"])
    p2_w, p2_b = f32(inp["p2_w"]), f32(inp["p2_b"])

    colsum = W.sum(axis=0)
    u1a = [upd_w1[r][:H] for r in range(2)]
    u1b = [upd_w1[r][H:] for r in range(2)]
    agw = [agg_w[r] @ u1b[r] for r in range(2)]
    biasp = [np.outer(colsum, agg_b[r] @ u1b[r]) + upd_b1[r][None, :] for r in range(2)]

    f = {}
    f["W"] = W
    f["wn1"] = wn @ agw[0]
    f["wn2"] = wn @ u1a[0]
    f["bx1"] = bn @ agw[0]
    f["ba1"] = bn @ u1a[0] + biasp[0]
    W_x2 = upd_w2[0] @ agw[1]          # [64,64]
    W_a2 = upd_w2[0] @ u1a[1]          # [64,64]
    # [128, 256]: cols 0:128 = X-path, 128:256 = A-path; block-diag over b-parity
    wxa2d = np.zeros((128, 256), np.float32)
    for s in range(2):
        wxa2d[s * 64:(s + 1) * 64, s * 64:(s + 1) * 64] = W_x2
        wxa2d[s * 64:(s + 1) * 64, 128 + s * 64:128 + (s + 1) * 64] = W_a2
    f["wxa2d"] = wxa2d
    f["add1"] = np.concatenate(
        [np.tile((upd_b2[0] @ agw[1])[None, :], (D, 1)),
         (upd_b2[0] @ u1a[1])[None, :] + biasp[1]], axis=1)
    p1r = p1_w.reshape(D, H, 2 * H)
    f["p1f"] = np.einsum("ph,jho->jpo", upd_w2[1], p1r)
    f["bt"] = p1_b + upd_b2[1] @ p1r.sum(axis=0)
    f["p2w"] = p2_w
    f["p2b"] = p2_b
    return f


# ------------------------------------------------------------- device kernel
def _build(flags):
    """flags: (has_bx1, has_ba1, has_add1) — whether tensor-bias adds are emitted."""
    has_bx1, has_ba1, has_add1 = flags
    nc = bacc.Bacc("TRN2", target_bir_lowering=False, debug=False)

    xsT_d = nc.dram_tensor("xsT", [D, BS], BF16, kind="ExternalInput").ap()
    wn1_d = nc.dram_tensor("wn1", [D, H], BF16, kind="ExternalInput").ap()
    wn2_d = nc.dram_tensor("wn2", [D, H], BF16, kind="ExternalInput").ap()
    W_d = nc.dram_tensor("Wm", [D, D], BF16, kind="ExternalInput").ap()
    wxa2d_d = nc.dram_tensor("wxa2d", [128, 256], BF16, kind="ExternalInput").ap()
    p1f_d = nc.dram_tensor("p1f", [2, 128, H, 2 * H], BF16, kind="ExternalInput").ap()
    bt_d = nc.dram_tensor("bt", [2 * H, 1], F32, kind="ExternalInput").ap()
    p2w_d = nc.dram_tensor("p2w", [2 * H, C], BF16, kind="ExternalInput").ap()
    p2b_d = nc.dram_tensor("p2b", [C, 1], F32, kind="ExternalInput").ap()
    bx1_d = nc.dram_tensor("bx1", [D, H], BF16, kind="ExternalInput").ap() if has_bx1 else None
    ba1_d = nc.dram_tensor("ba1", [D, H], BF16, kind="ExternalInput").ap() if has_ba1 else None
    add1_d = nc.dram_tensor("add1", [D, 2 * H], F32, kind="ExternalInput").ap() if has_add1 else None
    out_d = nc.dram_tensor("out", [BS, C], F32, kind="ExternalOutput").ap()

    with tile.TileContext(nc) as tc:
        with (
            tc.tile_pool(name="wp", bufs=1) as wp,
            tc.tile_pool(name="xt", bufs=3) as xt_pool,
            tc.tile_pool(name="xa0p", bufs=4) as xa0_pool,
            tc.tile_pool(name="xa1p", bufs=2) as xa1_pool,
            tc.tile_pool(name="pa0", bufs=2) as pa0_pool,
            tc.tile_pool(name="pa1", bufs=2) as pa1_pool,
            tc.tile_pool(name="pb", bufs=2) as pb_pool,
            tc.tile_pool(name="sm", bufs=2) as sm_pool,
            tc.tile_pool(name="ps_y", bufs=2, space="PSUM") as ps_y,
            tc.tile_pool(name="ps_mt", bufs=1, space="PSUM") as ps_mt,
            tc.tile_pool(name="ps_p", bufs=2, space="PSUM") as ps_p,
        ):
            # ---- replicated weights -> SBUF
            Wsb = wp.tile([128, 2, D], BF16)
            wn1sb = wp.tile([128, 2, H], BF16)
            wn2sb = wp.tile([128, 2, H], BF16)
            for ic in range(2):
                sl = slice(ic * 128, (ic + 1) * 128)
                nc.sync.dma_start(Wsb[:, ic, :], W_d[sl, :])
                nc.sync.dma_start(wn1sb[:, ic, :], wn1_d[sl, :])
                nc.sync.dma_start(wn2sb[:, ic, :], wn2_d[sl, :])
            wxa2sb = wp.tile([128, 256], BF16)
            nc.sync.dma_start(wxa2sb[:], wxa2d_d[:])
            p1fsb = []
            for jc in range(2):
                t_ = wp.tile([128, H, 2 * H], BF16, name=f"p1f{jc}")
                nc.sync.dma_start(t_[:], p1f_d[jc])
                p1fsb.append(t_)
            btsb = wp.tile([2 * H, 1], F32)
            nc.sync.dma_start(btsb[:], bt_d[:])
            p2wsb = wp.tile([2 * H, C], BF16)
            nc.sync.dma_start(p2wsb[:], p2w_d[:])
            p2bsb = wp.tile([C, 1], F32)
            nc.sync.dma_start(p2bsb[:], p2b_d[:])
            bx1sb = ba1sb = add1sb = None
            if has_bx1:
                bx1sb = wp.tile([128, 2, H], BF16)
                for ic in range(2):
                    nc.sync.dma_start(bx1sb[:, ic, :], bx1_d[ic * 128:(ic + 1) * 128, :])
            if has_ba1:
                ba1sb = wp.tile([128, 2, H], BF16)
                for ic in range(2):
                    nc.sync.dma_start(ba1sb[:, ic, :], ba1_d[ic * 128:(ic + 1) * 128, :])
            if has_add1:
                add1sb = wp.tile([128, 2, 2 * H], F32)
                for ic in range(2):
                    nc.sync.dma_start(add1sb[:, ic, :], add1_d[ic * 128:(ic + 1) * 128, :])
            identb = wp.tile([128, 128], BF16)
            masks.make_identity(nc, identb[:])

            xa0_tiles = {}

            def emit_front(t):
                """DMA x^T and build the rank-1 XA for b-tile t (DVE)."""
                b0 = t * PT
                xT = xt_pool.tile([128, 2, PT], BF16, tag="xT", name=f"xT_{t}")
                for ic in range(2):
                    nc.gpsimd.dma_start(xT[:, ic, :],
                                        xsT_d[ic * 128:(ic + 1) * 128, b0:b0 + PT])
                xa0 = []
                for ic in range(2):
                    a_ = xa0_pool.tile([128, PT, 2 * H], BF16, tag="xa0",
                                       name=f"xa0_{t}_{ic}")
                    for bs_ in range(2):
                        bsl = slice(bs_ * (PT // 2), (bs_ + 1) * (PT // 2))
                        nb = PT // 2
                        xbc = xT[:, ic, bsl].unsqueeze(2).broadcast_to([128, nb, H])
                        nc.vector.tensor_tensor(
                            a_[:, bsl, 0:H], xbc,
                            wn1sb[:, ic, :].unsqueeze(1).broadcast_to([128, nb, H]),
                            MULT)
                        nc.vector.tensor_tensor(
                            a_[:, bsl, H:2 * H], xbc,
                            wn2sb[:, ic, :].unsqueeze(1).broadcast_to([128, nb, H]),
                            MULT)
                        if has_bx1:
                            nc.vector.tensor_tensor(
                                a_[:, bsl, 0:H], a_[:, bsl, 0:H],
                                bx1sb[:, ic, :].unsqueeze(1).broadcast_to([128, nb, H]),
                                ADD)
                        if has_ba1:
                            nc.vector.tensor_tensor(
                                a_[:, bsl, H:2 * H], a_[:, bsl, H:2 * H],
                                ba1sb[:, ic, :].unsqueeze(1).broadcast_to([128, nb, H]),
                                ADD)
                    xa0.append(a_)
                xa0_tiles[t] = xa0

            pre1_pair = None
            emit_front(0)
            for t in range(NT):
                b0 = t * PT
                half = t % 2                     # position within a b-tile pair
                xa0 = xa0_tiles.pop(t)

                # ---- round 0: Y0 = W^T @ X1 (+A1 via identity), relu -> pre0
                pre0 = [pa0_pool.tile([128, PT, H], BF16, tag="pre0",
                                      name=f"pre0_{t}_{j}") for j in range(2)]
                for jc in range(2):
                    for bp2 in range(PT // 16):   # 16-b groups, 2 psum banks
                        py = ps_y.tile([128, 16, H], F32, tag="y0", bufs=1,
                                       name=f"py0_{t}_{jc}_{bp2}")
                        bsls = [slice(bp2 * 16 + g * 8, bp2 * 16 + (g + 1) * 8)
                                for g in range(2)]
                        psl = [py[:, g * 8:(g + 1) * 8, :] for g in range(2)]
                        # bank-interleaved: consecutive matmuls hit different banks
                        for ic in range(2):
                            for g in range(2):
                                nc.tensor.matmul(
                                    psl[g],
                                    Wsb[:, ic, jc * 128:(jc + 1) * 128],
                                    xa0[ic][:, bsls[g], 0:H],
                                    start=(ic == 0), stop=False,
                                    skip_group_check=True)
                        for g in range(2):
                            nc.tensor.matmul(
                                psl[g], identb[:], xa0[jc][:, bsls[g], H:2 * H],
                                start=False, stop=True, skip_group_check=True)
                        nc.scalar.activation(
                            pre0[jc][:, bp2 * 16:(bp2 + 1) * 16, :], py[:], RELU)

                # ---- T2: PE transpose pre0 [j,(2b,h)] -> pre0B [(s,h),(bp,j)]
                pre0B = pb_pool.tile([128, PT // 2, D], BF16, tag="preB",
                                     name=f"pre0B_{t}")
                for jc in range(2):
                    for gp in range(PT // 32):   # pairs of 8-bp groups
                        gs = [gp * 2, gp * 2 + 1]
                        pts = [ps_mt.tile([128, 8, 128], BF16, tag="mt",
                                          name=f"pt_{t}_{jc}_{g}") for g in gs]
                        for k_ in range(8):
                            for x, g in enumerate(gs):
                                bp = g * 8 + k_
                                nc.tensor.transpose(
                                    pts[x][:, k_, :],
                                    pre0[jc][:, bp * 2:(bp + 1) * 2, :], identb[:])
                        for x, g in enumerate(gs):
                            nc.scalar.copy(
                                pre0B[:, g * 8:(g + 1) * 8, jc * 128:(jc + 1) * 128],
                                pts[x][:])

                # ---- round 1, interleaved per 16-b group
                xa1 = [xa1_pool.tile([128, PT, H], BF16, tag="xa1",
                                     name=f"xa1_{t}_{j}") for j in range(2)]
                if half == 0:
                    pre1_pair = [pa1_pool.tile([128, 2 * PT, H], BF16, tag="pre1",
                                               name=f"pre1_{t}_{j}")
                                 for j in range(2)]
                for bp2 in range(PT // 16):      # 16 b's per stage
                    py1 = [ps_y.tile([128, 16, H], F32, tag="y1", bufs=2,
                                     name=f"py1_{t}_{jc}_{bp2}") for jc in range(2)]
                    # fused transpose-matmuls: X2 -> psum -> xa1 (SBUF),
                    # alternating between two banks
                    for jh in range(2):
                        jsl = slice(jh * 128, (jh + 1) * 128)
                        pmxs = [ps_mt.tile([128, 4, 2, H], F32, tag="mt",
                                           name=f"pmx_{t}_{bp2}_{jh}_{g}")
                                for g in range(2)]
                        for k_ in range(4):
                            for g in range(2):
                                bp = bp2 * 8 + g * 4 + k_
                                nc.tensor.matmul(
                                    pmxs[g][:, k_], pre0B[:, bp, jsl],
                                    wxa2sb[:, 0:128],
                                    start=True, stop=True)
                        for g in range(2):
                            nc.vector.tensor_copy(
                                xa1[jh][:, bp2 * 16 + g * 8:bp2 * 16 + (g + 1) * 8, :],
                                pmxs[g][:])
                    # W-mix first (start=True zeroes the bank), then the A-path
                    # fused matmuls accumulate into disjoint 2-b slices; emission
                    # interleaves (jc, g) so adjacent matmuls hit different banks.
                    for ic in range(2):
                        for jc in range(2):
                            for g in range(2):
                                bsl = slice(bp2 * 16 + g * 8, bp2 * 16 + (g + 1) * 8)
                                nc.tensor.matmul(
                                    py1[jc][:, g * 8:(g + 1) * 8, :],
                                    Wsb[:, ic, jc * 128:(jc + 1) * 128],
                                    xa1[ic][:, bsl, :],
                                    start=(ic == 0), stop=False,
                                    skip_group_check=True)
                    for k_ in range(4):
                        for jc in range(2):
                            jsl = slice(jc * 128, (jc + 1) * 128)
                            for g in range(2):
                                bp = bp2 * 8 + g * 4 + k_
                                pos = (g * 4 + k_) * 2
                                nc.tensor.matmul(
                                    py1[jc][:, pos:pos + 2, :],
                                    pre0B[:, bp, jsl], wxa2sb[:, 128:256],
                                    start=False, stop=(k_ == 3),
                                    skip_group_check=True)
                    for jc in range(2):
                        osl = slice(half * PT + bp2 * 16, half * PT + (bp2 + 1) * 16)
                        nc.scalar.activation(pre1_pair[jc][:, osl, :], py1[jc][:], RELU)

                if t + 1 < NT:
                    emit_front(t + 1)

                # ---- predictor head: once per b-tile pair
                if half == 1:
                    ppt = ps_p.tile([2 * H, 2 * PT], F32, tag="p", bufs=1,
                                    name=f"ppt_{t}")
                    for hp in range(H):
                        for jc in range(2):
                            nc.tensor.matmul(
                                ppt[:], p1fsb[jc][:, hp, :],
                                pre1_pair[jc][:, :, hp],
                                start=(hp == 0 and jc == 0),
                                stop=(hp == H - 1 and jc == 1))
                    tsb = sm_pool.tile([2 * H, 2 * PT], BF16, tag="t", name=f"tsb_{t}")
                    nc.scalar.activation(tsb[:], ppt[:], RELU, bias=btsb[:])
                    pc = ps_p.tile([2 * H, 2 * PT], F32, tag="p", bufs=1, name=f"pc_{t}")
                    nc.tensor.matmul(pc[:C, :], p2wsb[:], tsb[:], start=True, stop=True)
                    lsb = sm_pool.tile([C, 2 * PT], F32, tag="l", name=f"lsb_{t}")
                    nc.scalar.activation(lsb[:], pc[:C, :], IDENT, bias=p2bsb[:])
                    nc.gpsimd.dma_start(
                        out_d[b0 - PT:b0 + PT, :].rearrange("b c -> c b"), lsb[:])

    nc.compile()
    return nc


_cache = {}


def _program(flags):
    if flags not in _cache:
        _cache[flags] = _build(flags)
    return _cache[flags]


def _in_maps(inputs):
    f = _fold(inputs)
    has_bx1 = not np.allclose(f["bx1"], 0.0)
    has_ba1 = not np.allclose(f["ba1"], 0.0)
    has_add1 = not np.allclose(f["add1"], 0.0)
    flags = (has_bx1, has_ba1, has_add1)

    x = np.asarray(inputs["x"], dtype=np.float32)
    bf = lambda a: np.ascontiguousarray(np.asarray(a, dtype=np.float32),
                                        dtype=ml_dtypes.bfloat16)
    c32 = lambda a: np.ascontiguousarray(a, dtype=np.float32)

    common = {
        "wn1": bf(f["wn1"]),
        "wn2": bf(f["wn2"]),
        "Wm": bf(f["W"]),
        "wxa2d": bf(f["wxa2d"]),
        "p1f": bf(f["p1f"].reshape(2, 128, H, 2 * H)),
        "bt": c32(f["bt"].reshape(2 * H, 1)),
        "p2w": bf(f["p2w"]),
        "p2b": c32(f["p2b"].reshape(C, 1)),
    }
    if has_bx1:
        common["bx1"] = bf(f["bx1"])
    if has_ba1:
        common["ba1"] = bf(f["ba1"])
    if has_add1:
        common["add1"] = c32(f["add1"])

    maps = []
    for c in range(NCORES):
        xs = x[c * BS:(c + 1) * BS]                 # [BS, D]
        m = dict(common)
        m["xsT"] = bf(xs.T)                          # [D, BS] bf16
        maps.append(m)
    return flags, maps


def run(inputs, trace=False, **kw):
    flags, maps = _in_maps(inputs)
    nc = _program(flags)
    res = bass_utils.run_bass_kernel_spmd(
        nc, maps, core_ids=list(range(NCORES)), trace=trace, **kw)
    out = np.concatenate([r["out"] for r in res.results], axis=0)
    return out.astype(np.float32), res


def kernel(**inputs):
    out, _ = run(inputs)
    return out


# revision 23
# speedup vs baseline: 1.4389x; 1.4389x over previous
"""Trainium2 Bass kernel for nn_CausalBoostNet (gnn_message_passing).

Strategy
--------
Data-parallel over batch B=4096 across 8 NeuronCores (512 rows each). All
small parameters are replicated. The whole network is algebraically folded
on the host so the device executes, per 64-row batch tile:

  round 0:  X1 = x (*) wn1, A1 = x (*) wn2             (rank-1, DVE, bf16)
            Y0 = W^T @ X1  (+ A1 via identity matmul)  (PE)
            pre0 = relu(Y0 + A1)                       (ACT, -> bf16)
  T2:       pre0 [j,(b,h)] -> stacked [(b%2,h), (b/2,j)] (PE transpose)
  round 1:  XA = pre0 @ blockdiag[W_x2|W_a2]           (fused transpose-matmul,
            Y1 = W^T @ X2 (+ A2), pre1 = relu(.)        K=128, both b-parities)
  head:     logits = relu(pre1 . p1f + bt) @ p2w + p2b (PE, two tiles at once)

where wn1 = wn @ (agg_w0 @ u1b0), wn2 = wn @ u1a0, W_x2 = u2_0 @ (agg_w1 @ u1b1),
W_a2 = u2_0 @ u1a1, p1f[j,hp,o] = sum_h u2_1[hp,h] p1_w[(j,h),o], etc.
All per-feature biases (zeros in this problem's init) are folded into
rank-1 addends / per-partition activation biases; the emission of the
(tensor) bias adds is skipped when the host-folded values are exactly zero.
"""

import numpy as np
import ml_dtypes

import concourse.bacc as bacc
import concourse.bass as bass
import concourse.mybir as mybir
import concourse.tile as tile
from concourse import bass_utils, masks

# problem constants
B, D, H, C = 4096, 256, 64, 10
NCORES = 8
BS = B // NCORES          # per-core batch shard
PT = 64                   # batch tile
NT = BS // PT
TAU = 0.3

F32 = mybir.dt.float32
BF16 = mybir.dt.bfloat16
RELU = mybir.ActivationFunctionType.Relu
IDENT = mybir.ActivationFunctionType.Identity
MULT = mybir.AluOpType.mult
ADD = mybir.AluOpType.add


# ----------------------------------------------------------------- host math
def _get_W_np(A_raw, priority, edge_logits, tau=TAU):
    k = priority.shape[0]
    d = A_raw.shape[0]
    A = A_raw - A_raw.max(axis=1, keepdims=True)
    A = np.exp(A)
    A = A / A.sum(axis=1, keepdims=True)
    direction = 1.0 / (1.0 + np.exp(-(priority[None, :] - priority[:, None]) / tau))
    edges = (edge_logits > 0.0).astype(A.dtype)
    S = edges * direction * (1.0 - np.eye(k, dtype=A.dtype))
    W = (A @ S @ A.T) * (1.0 - np.eye(d, dtype=A.dtype))
    return W


def _fold(inp):
    f32 = lambda a: np.asarray(a, dtype=np.float32)
    wn, bn = f32(inp["wn"]), f32(inp["bn"])
    W = _get_W_np(f32(inp["A_raw"]), f32(inp["priority"]), f32(inp["edge_logits"]))
    agg_w, agg_b = f32(inp["agg_w"]), f32(inp["agg_b"])
    upd_w1, upd_b1 = f32(inp["upd_w1"]), f32(inp["upd_b1"])
    upd_w2, upd_b2 = f32(inp["upd_w2"]), f32(inp["upd_b2"])
    p1_w, p1_b = f32(inp["p1_w"]), f32(inp["p1_b"])
    p2_w, p2_b = f32(inp["p2_w"]), f32(inp["p2_b"])

    colsum = W.sum(axis=0)
    u1a = [upd_w1[r][:H] for r in range(2)]
    u1b = [upd_w1[r][H:] for r in range(2)]
    agw = [agg_w[r] @ u1b[r] for r in range(2)]
    biasp = [np.outer(colsum, agg_b[r] @ u1b[r]) + upd_b1[r][None, :] for r in range(2)]

    f = {}
    f["W"] = W
    f["wn1"] = wn @ agw[0]
    f["wn2"] = wn @ u1a[0]
    f["bx1"] = bn @ agw[0]
    f["ba1"] = bn @ u1a[0] + biasp[0]
    W_x2 = upd_w2[0] @ agw[1]          # [64,64]
    W_a2 = upd_w2[0] @ u1a[1]          # [64,64]
    # [128, 256]: cols 0:128 = X-path, 128:256 = A-path; block-diag over b-parity
    wxa2d = np.zeros((128, 256), np.float32)
    for s in range(2):
        wxa2d[s * 64:(s + 1) * 64, s * 64:(s + 1) * 64] = W_x2
        wxa2d[s * 64:(s + 1) * 64, 128 + s * 64:128 + (s + 1) * 64] = W_a2
    f["wxa2d"] = wxa2d
    f["add1"] = np.concatenate(
        [np.tile((upd_b2[0] @ agw[1])[None, :], (D, 1)),
         (upd_b2[0] @ u1a[1])[None, :] + biasp[1]], axis=1)
    p1r = p1_w.reshape(D, H, 2 * H)
    f["p1f"] = np.einsum("ph,jho->jpo", upd_w2[1], p1r)
    f["bt"] = p1_b + upd_b2[1] @ p1r.sum(axis=0)
    f["p2w"] = p2_w
    f["p2b"] = p2_b
    return f


# ------------------------------------------------------------- device kernel
def _build(flags):
    """flags: (has_bx1, has_ba1, has_add1) — whether tensor-bias adds are emitted."""
    has_bx1, has_ba1, has_add1 = flags
    nc = bacc.Bacc("TRN2", target_bir_lowering=False, debug=False)

    xsT_d = nc.dram_tensor("xsT", [D, BS], BF16, kind="ExternalInput").ap()
    wn1_d = nc.dram_tensor("wn1", [D, H], BF16, kind="ExternalInput").ap()
    wn2_d = nc.dram_tensor("wn2", [D, H], BF16, kind="ExternalInput").ap()
    W_d = nc.dram_tensor("Wm", [D, D], BF16, kind="ExternalInput").ap()
    wxa2d_d = nc.dram_tensor("wxa2d", [128, 256], BF16, kind="ExternalInput").ap()
    p1f_d = nc.dram_tensor("p1f", [2, 128, H, 2 * H], BF16, kind="ExternalInput").ap()
    bt_d = nc.dram_tensor("bt", [2 * H, 1], F32, kind="ExternalInput").ap()
    p2w_d = nc.dram_tensor("p2w", [2 * H, C], BF16, kind="ExternalInput").ap()
    p2b_d = nc.dram_tensor("p2b", [C, 1], F32, kind="ExternalInput").ap()
    bx1_d = nc.dram_tensor("bx1", [D, H], BF16, kind="ExternalInput").ap() if has_bx1 else None
    ba1_d = nc.dram_tensor("ba1", [D, H], BF16, kind="ExternalInput").ap() if has_ba1 else None
    add1_d = nc.dram_tensor("add1", [D, 2 * H], F32, kind="ExternalInput").ap() if has_add1 else None
    out_d = nc.dram_tensor("out", [BS, C], F32, kind="ExternalOutput").ap()

    with tile.TileContext(nc) as tc:
        with (
            tc.tile_pool(name="wp", bufs=1) as wp,
            tc.tile_pool(name="xt", bufs=3) as xt_pool,
            tc.tile_pool(name="xa0p", bufs=4) as xa0_pool,
            tc.tile_pool(name="xa1p", bufs=2) as xa1_pool,
            tc.tile_pool(name="pa0", bufs=2) as pa0_pool,
            tc.tile_pool(name="pa1", bufs=2) as pa1_pool,
            tc.tile_pool(name="pb", bufs=2) as pb_pool,
            tc.tile_pool(name="sm", bufs=2) as sm_pool,
            tc.tile_pool(name="ps_y", bufs=2, space="PSUM") as ps_y,
            tc.tile_pool(name="ps_mt", bufs=2, space="PSUM") as ps_mt,
            tc.tile_pool(name="ps_p", bufs=2, space="PSUM") as ps_p,
        ):
            # ---- replicated weights -> SBUF
            Wsb = wp.tile([128, 2, D], BF16)
            wn1sb = wp.tile([128, 2, H], BF16)
            wn2sb = wp.tile([128, 2, H], BF16)
            for ic in range(2):
                sl = slice(ic * 128, (ic + 1) * 128)
                nc.sync.dma_start(Wsb[:, ic, :], W_d[sl, :])
                nc.sync.dma_start(wn1sb[:, ic, :], wn1_d[sl, :])
                nc.sync.dma_start(wn2sb[:, ic, :], wn2_d[sl, :])
            wxa2sb = wp.tile([128, 256], BF16)
            nc.sync.dma_start(wxa2sb[:], wxa2d_d[:])
            p1fsb = []
            for jc in range(2):
                t_ = wp.tile([128, H, 2 * H], BF16, name=f"p1f{jc}")
                nc.sync.dma_start(t_[:], p1f_d[jc])
                p1fsb.append(t_)
            btsb = wp.tile([2 * H, 1], F32)
            nc.sync.dma_start(btsb[:], bt_d[:])
            p2wsb = wp.tile([2 * H, C], BF16)
            nc.sync.dma_start(p2wsb[:], p2w_d[:])
            p2bsb = wp.tile([C, 1], F32)
            nc.sync.dma_start(p2bsb[:], p2b_d[:])
            bx1sb = ba1sb = add1sb = None
            if has_bx1:
                bx1sb = wp.tile([128, 2, H], BF16)
                for ic in range(2):
                    nc.sync.dma_start(bx1sb[:, ic, :], bx1_d[ic * 128:(ic + 1) * 128, :])
            if has_ba1:
                ba1sb = wp.tile([128, 2, H], BF16)
                for ic in range(2):
                    nc.sync.dma_start(ba1sb[:, ic, :], ba1_d[ic * 128:(ic + 1) * 128, :])
            if has_add1:
                add1sb = wp.tile([128, 2, 2 * H], F32)
                for ic in range(2):
                    nc.sync.dma_start(add1sb[:, ic, :], add1_d[ic * 128:(ic + 1) * 128, :])
            identb = wp.tile([128, 128], BF16)
            masks.make_identity(nc, identb[:])

            xa0_tiles = {}

            def emit_front(t):
                """DMA x^T and build the rank-1 XA for b-tile t (DVE)."""
                b0 = t * PT
                xT = xt_pool.tile([128, 2, PT], BF16, tag="xT", name=f"xT_{t}")
                for ic in range(2):
                    nc.gpsimd.dma_start(xT[:, ic, :],
                                        xsT_d[ic * 128:(ic + 1) * 128, b0:b0 + PT])
                xa0 = []
                for ic in range(2):
                    a_ = xa0_pool.tile([128, PT, 2 * H], BF16, tag="xa0",
                                       name=f"xa0_{t}_{ic}")
                    for bs_ in range(2):
                        bsl = slice(bs_ * (PT // 2), (bs_ + 1) * (PT // 2))
                        nb = PT // 2
                        xbc = xT[:, ic, bsl].unsqueeze(2).broadcast_to([128, nb, H])
                        nc.vector.tensor_tensor(
                            a_[:, bsl, 0:H], xbc,
                            wn1sb[:, ic, :].unsqueeze(1).broadcast_to([128, nb, H]),
                            MULT)
                        nc.vector.tensor_tensor(
                            a_[:, bsl, H:2 * H], xbc,
                            wn2sb[:, ic, :].unsqueeze(1).broadcast_to([128, nb, H]),
                            MULT)
                        if has_bx1:
                            nc.vector.tensor_tensor(
                                a_[:, bsl, 0:H], a_[:, bsl, 0:H],
                                bx1sb[:, ic, :].unsqueeze(1).broadcast_to([128, nb, H]),
                                ADD)
                        if has_ba1:
                            nc.vector.tensor_tensor(
                                a_[:, bsl, H:2 * H], a_[:, bsl, H:2 * H],
                                ba1sb[:, ic, :].unsqueeze(1).broadcast_to([128, nb, H]),
                                ADD)
                    xa0.append(a_)
                xa0_tiles[t] = xa0

            pre1_pair = None
            emit_front(0)
            for t in range(NT):
                b0 = t * PT
                half = t % 2                     # position within a b-tile pair
                xa0 = xa0_tiles.pop(t)

                # ---- round 0: Y0 = W^T @ X1 (+A1 via identity), relu -> pre0
                pre0 = [pa0_pool.tile([128, PT, H], BF16, tag="pre0",
                                      name=f"pre0_{t}_{j}") for j in range(2)]
                for jc in range(2):
                    for bp2 in range(PT // 16):   # 16-b groups, 2 psum banks
                        py = ps_y.tile([128, 16, H], F32, tag="y0", bufs=1,
                                       name=f"py0_{t}_{jc}_{bp2}")
                        bsls = [slice(bp2 * 16 + g * 8, bp2 * 16 + (g + 1) * 8)
                                for g in range(2)]
                        psl = [py[:, g * 8:(g + 1) * 8, :] for g in range(2)]
                        # bank-interleaved: consecutive matmuls hit different banks
                        for ic in range(2):
                            for g in range(2):
                                nc.tensor.matmul(
                                    psl[g],
                                    Wsb[:, ic, jc * 128:(jc + 1) * 128],
                                    xa0[ic][:, bsls[g], 0:H],
                                    start=(ic == 0), stop=False,
                                    skip_group_check=True)
                        for g in range(2):
                            nc.tensor.matmul(
                                psl[g], identb[:], xa0[jc][:, bsls[g], H:2 * H],
                                start=False, stop=True, skip_group_check=True)
                        nc.scalar.activation(
                            pre0[jc][:, bp2 * 16:(bp2 + 1) * 16, :], py[:], RELU)

                # ---- T2: PE transpose pre0 [j,(2b,h)] -> pre0B [(s,h),(bp,j)]
                pre0B = pb_pool.tile([128, PT // 2, D], BF16, tag="preB",
                                     name=f"pre0B_{t}")
                for jc in range(2):
                    for gp in range(PT // 32):   # pairs of 8-bp groups
                        gs = [gp * 2, gp * 2 + 1]
                        pts = [ps_mt.tile([128, 8, 128], BF16, tag="mt",
                                          name=f"pt_{t}_{jc}_{g}") for g in gs]
                        for k_ in range(8):
                            for x, g in enumerate(gs):
                                bp = g * 8 + k_
                                nc.tensor.transpose(
                                    pts[x][:, k_, :],
                                    pre0[jc][:, bp * 2:(bp + 1) * 2, :], identb[:])
                        for x, g in enumerate(gs):
                            nc.scalar.copy(
                                pre0B[:, g * 8:(g + 1) * 8, jc * 128:(jc + 1) * 128],
                                pts[x][:])

                # ---- round 1, interleaved per 16-b group
                xa1 = [xa1_pool.tile([128, PT, H], BF16, tag="xa1",
                                     name=f"xa1_{t}_{j}") for j in range(2)]
                if half == 0:
                    pre1_pair = [pa1_pool.tile([128, 2 * PT, H], BF16, tag="pre1",
                                               name=f"pre1_{t}_{j}")
                                 for j in range(2)]
                for bp2 in range(PT // 16):      # 16 b's per stage
                    py1 = [ps_y.tile([128, 16, H], F32, tag="y1", bufs=1,
                                     name=f"py1_{t}_{jc}_{bp2}") for jc in range(2)]
                    # fused transpose-matmuls: X2 -> psum -> xa1 (SBUF),
                    # alternating between two banks
                    for jh in range(2):
                        jsl = slice(jh * 128, (jh + 1) * 128)
                        pmxs = [ps_mt.tile([128, 4, 2, H], F32, tag="mt",
                                           name=f"pmx_{t}_{bp2}_{jh}_{g}")
                                for g in range(2)]
                        for k_ in range(4):
                            for g in range(2):
                                bp = bp2 * 8 + g * 4 + k_
                                nc.tensor.matmul(
                                    pmxs[g][:, k_], pre0B[:, bp, jsl],
                                    wxa2sb[:, 0:128],
                                    start=True, stop=True)
                        for g in range(2):
                            nc.vector.tensor_copy(
                                xa1[jh][:, bp2 * 16 + g * 8:bp2 * 16 + (g + 1) * 8, :],
                                pmxs[g][:])
                    # W-mix first (start=True zeroes the bank), then the A-path
                    # fused matmuls accumulate into disjoint 2-b slices; emission
                    # interleaves (jc, g) so adjacent matmuls hit different banks.
                    for ic in range(2):
                        for jc in range(2):
                            for g in range(2):
                                bsl = slice(bp2 * 16 + g * 8, bp2 * 16 + (g + 1) * 8)
                                nc.tensor.matmul(
                                    py1[jc][:, g * 8:(g + 1) * 8, :],
                                    Wsb[:, ic, jc * 128:(jc + 1) * 128],
                                    xa1[ic][:, bsl, :],
                                    start=(ic == 0), stop=False,
                                    skip_group_check=True)
                    for k_ in range(4):
                        for jc in range(2):
                            jsl = slice(jc * 128, (jc + 1) * 128)
                            for g in range(2):
                                bp = bp2 * 8 + g * 4 + k_
                                pos = (g * 4 + k_) * 2
                                nc.tensor.matmul(
                                    py1[jc][:, pos:pos + 2, :],
                                    pre0B[:, bp, jsl], wxa2sb[:, 128:256],
                                    start=False, stop=(k_ == 3),
                                    skip_group_check=True)
                    for jc in range(2):
                        osl = slice(half * PT + bp2 * 16, half * PT + (bp2 + 1) * 16)
                        nc.scalar.activation(pre1_pair[jc][:, osl, :], py1[jc][:], RELU)

                if t + 1 < NT:
                    emit_front(t + 1)

                # ---- predictor head: once per b-tile pair
                if half == 1:
                    ppts = [ps_p.tile([2 * H, 2 * PT], F32, tag="p",
                                      name=f"ppt_{t}_{a}") for a in range(2)]
                    for hp in range(H):
                        for jc in range(2):
                            a = hp % 2
                            nc.tensor.matmul(
                                ppts[a][:], p1fsb[jc][:, hp, :],
                                pre1_pair[jc][:, :, hp],
                                start=(hp < 2 and jc == 0),
                                stop=(hp >= H - 2 and jc == 1),
                                skip_group_check=True)
                    tq = sm_pool.tile([2 * H, 2 * PT], F32, tag="tq", name=f"tq_{t}")
                    nc.scalar.copy(tq[:], ppts[1][:])
                    tq2 = sm_pool.tile([2 * H, 2 * PT], F32, tag="tq2", name=f"tq2_{t}")
                    nc.vector.tensor_tensor(tq2[:], ppts[0][:], tq[:], ADD)
                    tsb = sm_pool.tile([2 * H, 2 * PT], BF16, tag="t", name=f"tsb_{t}")
                    nc.scalar.activation(tsb[:], tq2[:], RELU, bias=btsb[:])
                    pc = ps_p.tile([2 * H, 2 * PT], F32, tag="p", name=f"pc_{t}")
                    nc.tensor.matmul(pc[:C, :], p2wsb[:], tsb[:], start=True, stop=True)
                    lsb = sm_pool.tile([C, 2 * PT], F32, tag="l", name=f"lsb_{t}")
                    nc.scalar.activation(lsb[:], pc[:C, :], IDENT, bias=p2bsb[:])
                    nc.gpsimd.dma_start(
                        out_d[b0 - PT:b0 + PT, :].rearrange("b c -> c b"), lsb[:])

    nc.compile()
    return nc


_cache = {}


def _program(flags):
    if flags not in _cache:
        _cache[flags] = _build(flags)
    return _cache[flags]


def _in_maps(inputs):
    f = _fold(inputs)
    has_bx1 = not np.allclose(f["bx1"], 0.0)
    has_ba1 = not np.allclose(f["ba1"], 0.0)
    has_add1 = not np.allclose(f["add1"], 0.0)
    flags = (has_bx1, has_ba1, has_add1)

    x = np.asarray(inputs["x"], dtype=np.float32)
    bf = lambda a: np.ascontiguousarray(np.asarray(a, dtype=np.float32),
                                        dtype=ml_dtypes.bfloat16)
    c32 = lambda a: np.ascontiguousarray(a, dtype=np.float32)

    common = {
        "wn1": bf(f["wn1"]),
        "wn2": bf(f["wn2"]),
        "Wm": bf(f["W"]),
        "wxa2d": bf(f["wxa2d"]),
        "p1f": bf(f["p1f"].reshape(2, 128, H, 2 * H)),
        "bt": c32(f["bt"].reshape(2 * H, 1)),
        "p2w": bf(f["p2w"]),
        "p2b": c32(f["p2b"].reshape(C, 1)),
    }
    if has_bx1:
        common["bx1"] = bf(f["bx1"])
    if has_ba1:
        common["ba1"] = bf(f["ba1"])
    if has_add1:
        common["add1"] = c32(f["add1"])

    maps = []
    for c in range(NCORES):
        xs = x[c * BS:(c + 1) * BS]                 # [BS, D]
        m = dict(common)
        m["xsT"] = bf(xs.T)                          # [D, BS] bf16
        maps.append(m)
    return flags, maps


def run(inputs, trace=False, **kw):
    flags, maps = _in_maps(inputs)
    nc = _program(flags)
    res = bass_utils.run_bass_kernel_spmd(
        nc, maps, core_ids=list(range(NCORES)), trace=trace, **kw)
    out = np.concatenate([r["out"] for r in res.results], axis=0)
    return out.astype(np.float32), res


def kernel(**inputs):
    out, _ = run(inputs)
    return out
